# revision 20
# baseline (speedup 1.0000x reference)
"""Trainium2 Bass kernel for BowEncoder (embedding gather + masked mean pool).

Reference computation:
    out[b, :] = (1/len_b) * sum_{t < len_b} emb[input[b, t], :]
with input [64, 2048] int, input_lens [64] int, emb [50257, 256] f32.

Strategy: data-parallel over batch across 8 NeuronCores (8 rows per core).
Each core:
  - loads one int32 "meta" tile: transposed indices [128, 128] (column
    b*16+k is one 128-token tile), lens replicated across partitions
    [128, 8], and an iota constant [128, 16],
  - builds probs[p, b*16+k] = (128k + p < len_b) ? 1/len_b : 0 on device
    (is_lt + reciprocal + mult on DVE),
  - indirect-DMA gathers embedding rows, ROWS_PER_GATHER batch rows per
    gather instruction (fewer SWDGE instructions -> fewer DMA sem lanes,
    which the final Drain instruction must wait on),
  - pools each 128-token tile with a PE matmul probs_col.T @ gathered_tile
    accumulated in a per-batch PSUM bank [1, 256] (1/len folded into probs),
  - copies each PSUM row into one single-partition SBUF tile [1, 8*256]
    and writes the whole core output with a single DMA.

Hardware quirk driving the structure: this walrus build allows only ONE
sync-wait on Matmult and DMACopy instructions and a limited number on the
kernel-tail Drain. Hence: a dummy PE matmul absorbs the "probs ready" DVE
wait; pool buffer counts are sized so no tile slot is ever reused (no
second wait); and the instruction mix keeps the number of distinct
semaphore lanes small.
"""

import numpy as np

import concourse.bass as bass
import concourse.mybir as mybir
import concourse.tile as tile
from concourse.bass_utils import run_bass_kernel_spmd

P = 128
B, T, V, H = 64, 2048, 50257, 256
NCORES = 8
BPC = B // NCORES          # batch rows per core
NT = T // P                # 128-token tiles per batch row
NCOL = BPC * NT            # index/prob columns per core
ROWS_PER_GATHER = 2        # batch rows per indirect-DMA gather
NGATH = BPC // ROWS_PER_GATHER
GCOL = ROWS_PER_GATHER * NT  # index columns per gather

_DT = mybir.dt


def _split_multi_waits(nc: bass.Bass, max_waits: int = 1) -> None:
    """This walrus build rejects instructions carrying more than one
    sync-wait ("Too many sync wait commands"). Hoist excess waits onto
    same-engine NoOps inserted immediately before the instruction —
    engine queues execute in order, so the waits still gate it."""
    for fn in nc.m.functions:
        for bb in fn.blocks:
            rebuilt = []
            changed = False
            for inst in bb.instructions:
                si = inst.sync_info
                if si is not None and si.on_wait and len(si.on_wait) > max_waits:
                    waits = list(si.on_wait)
                    extra, keep = waits[:-max_waits], waits[-max_waits:]
                    for j in range(0, len(extra), max_waits):
                        rebuilt.append(
                            mybir.InstNoOp(
                                name=f"{inst.name}-wsplit{j}",
                                sync_info=mybir.SyncInfo(
                                    on_wait=extra[j : j + max_waits], on_update=[]
                                ),
                                bass_nofuse=True,
                                engine=inst.engine,
                            )
                        )
                    inst.sync_info = mybir.SyncInfo(
                        on_wait=keep, on_update=list(si.on_update or [])
                    )
                    changed = True
                rebuilt.append(inst)
            if changed:
                bb.instructions = rebuilt


def _build_nc() -> bass.Bass:
    nc = bass.Bass("TRN2", target_bir_lowering=False)

    # meta = [idx | lens_rep | iota]: int32 [128, 128 + 8 + 16]
    meta = nc.dram_tensor("meta", [P, NCOL + BPC + NT], _DT.int32, kind="ExternalInput")
    emb = nc.dram_tensor("emb", [V, H], _DT.float32, kind="ExternalInput")
    out = nc.dram_tensor("out", [BPC, H], _DT.float32, kind="ExternalOutput")

    with tile.TileContext(nc) as tc:
        with (
            tc.tile_pool(name="const", bufs=1) as const,
            tc.tile_pool(name="gath", bufs=32) as gpool,
            tc.tile_pool(name="outp", bufs=1) as opool,
            tc.tile_pool(name="psum", bufs=1, space="PSUM") as psum_tp,
        ):
            meta_sb = const.tile([P, NCOL + BPC + NT], _DT.int32)
            nc.sync.dma_start(out=meta_sb[:], in_=meta[:, :])
            idx_sb = meta_sb[:, :NCOL]

            lens_f = const.tile([P, BPC], _DT.float32)
            nc.vector.tensor_copy(out=lens_f[:], in_=meta_sb[:, NCOL : NCOL + BPC])
            recip = const.tile([P, BPC], _DT.float32)
            nc.vector.reciprocal(out=recip[:], in_=lens_f[:])
            iota_f = const.tile([P, NT], _DT.float32)
            nc.vector.tensor_copy(
                out=iota_f[:], in_=meta_sb[:, NCOL + BPC : NCOL + BPC + NT]
            )

            # ---- probs[p, b*NT+k] = (iota < len_b) * (1/len_b)
            probs = const.tile([P, NCOL], _DT.float32)
            for b in range(BPC):
                sl = slice(b * NT, (b + 1) * NT)
                nc.vector.tensor_tensor(
                    out=probs[:, sl],
                    in0=iota_f[:],
                    in1=lens_f[:, b : b + 1].to_broadcast([P, NT]),
                    op=mybir.AluOpType.is_lt,
                )
                nc.vector.tensor_tensor(
                    out=probs[:, sl],
                    in0=probs[:, sl],
                    in1=recip[:, b : b + 1].to_broadcast([P, NT]),
                    op=mybir.AluOpType.mult,
                )

            # ---- dummy PE op into batch 0's accumulator bank: absorbs the
            #      "probs ready" DVE wait on the PE clock so per-batch
            #      matmuls carry only their gather-DMA wait. The real k=0
            #      matmul restarts the bank (start=True), discarding this.
            accs = []
            for b in range(BPC):
                acc = psum_tp.tile(
                    [1, H], _DT.float32, space="PSUM", tag=f"acc{b}", name=f"acc{b}"
                )
                accs.append(acc)
            nc.tensor.matmul(
                out=accs[0][0:1, 0:NCOL],
                lhsT=probs[:, 0:1],
                rhs=probs[:, :],
                start=True,
                stop=True,
            )

            # ---- main loop: one indirect gather per 128-token tile (the HW
            #      DGE consumes exactly one offset per dest partition, so
            #      each gather is [128,1] offsets -> [128, 256] rows), then
            #      one accumulating PE matmul per tile into per-batch banks.
            cat = opool.tile([1, BPC * H], _DT.float32)
            for b in range(BPC):
                for k in range(NT):
                    col = b * NT + k
                    gath = gpool.tile([P, H], _DT.float32, tag="gath")
                    nc.gpsimd.indirect_dma_start(
                        out=gath[:],
                        out_offset=None,
                        in_=emb[:],
                        in_offset=bass.IndirectOffsetOnAxis(
                            ap=idx_sb[:, col : col + 1], axis=0
                        ),
                    )
                    nc.tensor.matmul(
                        out=accs[b][:],
                        lhsT=probs[:, col : col + 1],
                        rhs=gath[:],
                        start=(k == 0),
                        stop=(k == NT - 1),
                    )
                nc.vector.tensor_copy(
                    out=cat[0:1, b * H : (b + 1) * H], in_=accs[b][:]
                )
            # single out DMA: [1, 8*256] SBUF row -> [8, 256] DRAM (same
            # linear element order on both sides)
            nc.sync.dma_start(out=out[:, :], in_=cat[:])

    _split_multi_waits(nc)
    return nc


def _prep_in_maps(input_ids: np.ndarray, input_lens: np.ndarray, emb: np.ndarray):
    input_ids = np.ascontiguousarray(input_ids.astype(np.int32))
    input_lens = np.ascontiguousarray(input_lens.astype(np.int32))
    emb = np.ascontiguousarray(emb.astype(np.float32))
    iota = (np.arange(P)[:, None] + P * np.arange(NT)[None, :]).astype(np.int32)
    in_maps = []
    for c in range(NCORES):
        rows = input_ids[c * BPC : (c + 1) * BPC]              # [8, 2048]
        # idx_t[p, b*NT+k] = rows[b, k*128+p]
        idx_t = rows.reshape(BPC, NT, P).transpose(2, 0, 1).reshape(P, NCOL)
        lens_rep = np.broadcast_to(
            input_lens[c * BPC : (c + 1) * BPC][None, :], (P, BPC)
        )
        meta = np.ascontiguousarray(
            np.concatenate([idx_t, lens_rep, iota], axis=1)
        )
        in_maps.append({"meta": meta, "emb": emb})
    return in_maps


_CACHE: dict = {}


def _run(inputs: dict, trace: bool = False):
    if "nc" not in _CACHE:
        _CACHE["nc"] = _build_nc()
    nc = _CACHE["nc"]
    in_maps = _prep_in_maps(inputs["input"], inputs["input_lens"], inputs["emb"])
    res = run_bass_kernel_spmd(nc, in_maps, core_ids=list(range(NCORES)), trace=trace)
    out = np.concatenate([res.results[c]["out"] for c in range(NCORES)], axis=0)
    return out.astype(np.float32), res


def kernel(input: np.ndarray, input_lens: np.ndarray, emb: np.ndarray) -> np.ndarray:
    out, _ = _run({"input": input, "input_lens": input_lens, "emb": emb})
    return out
